# revision 4
# baseline (speedup 1.0000x reference)
"""HalfKA NNUE forward pass on 8 Trainium2 NeuronCores — sparse gather version.

Network (fp32 reference):
    h1  = relu(x @ W1.T + b1)     x:[2048, 98304] sparse 0/1 (~32 nnz/row), W1:[256, 98304]
    h2  = relu(h1 @ W2.T + b2)    W2:[32, 256]
    out = h2 @ Wout.T + bout      Wout:[1, 32]  -> [2048, 1]

Strategy: data-parallel over the batch. Each core handles 256 rows as two
groups of 128. Instead of streaming the dense x (100 MB/core), the host
extracts the active-feature indices; the device gathers only the needed W1.T
rows (bf16, ~4 MB/core) with gpsimd.dma_gather and contracts them against a
host-built 0/1 selection matrix xc on the PE:

    h1[b, :] = sum_u xc[u, b] * W1T[U[u], :]      (U = union of the group's
                                                   active features)

dma_gather uses int16 indices (max 32767 < 98304), so each group's union is
split into 3 windows of 32768 rows, one gather call per window with a
base-offset view of the table. Pad slots point at the window's row 0; the
zeros in xc kill their contribution. fc2/fc3 are tiny and computed per group.
No collectives: each core writes its own 256 outputs.
"""

import sys

sys.path.insert(0, "/opt/trn_rl_repo")

from contextlib import ExitStack

import numpy as np
import ml_dtypes

import concourse.bass as bass
import concourse.tile as tile
from concourse import bacc, mybir
from concourse.bass_utils import run_bass_kernel_spmd

f32 = mybir.dt.float32
bf16 = mybir.dt.bfloat16
i16 = mybir.dt.int16

N_CORES = 8
B = 2048
IN_DIM = 98304
H1 = 256
H2 = 32
P = 128

RPC = B // N_CORES               # 256 rows per core
NG = RPC // P                    # 2 groups of 128 rows per core
NWIN = 3                         # int16 index windows over IN_DIM
WIN = 32768
NIDX_W = 1536                    # gathered-index capacity per (group, window)
SLOTS_W = NIDX_W // P            # 12 k-slots per window
T = NWIN * SLOTS_W               # 36 k-tiles per group
M_T = H1 // P                    # 2 h1 partition-tiles

_CACHED = {}


def _build_program(nidx_w=NIDX_W):
    slots_w = nidx_w // P
    t_tiles = NWIN * slots_w

    nc = bacc.Bacc(
        "TRN2",
        target_bir_lowering=False,
        debug=False,
        num_devices=N_CORES,
    )

    w1t = nc.dram_tensor("w1t", [IN_DIM, H1], bf16, kind="ExternalInput")
    idxs = nc.dram_tensor("idxs", [P, NG, NWIN, nidx_w // 16], i16, kind="ExternalInput")
    xc = nc.dram_tensor("xc", [P, NG, t_tiles, P], bf16, kind="ExternalInput")
    b1 = nc.dram_tensor("b1", [P, M_T], f32, kind="ExternalInput")
    w2t = nc.dram_tensor("w2t", [P, M_T, H2], f32, kind="ExternalInput")
    b2 = nc.dram_tensor("b2", [H2, 1], f32, kind="ExternalInput")
    woutt = nc.dram_tensor("woutt", [H2 + 1, 1], f32, kind="ExternalInput")
    ident = nc.dram_tensor("ident", [P, P], f32, kind="ExternalInput")
    out = nc.dram_tensor("out", [NG, P], f32, kind="ExternalOutput")

    with tile.TileContext(nc) as tc:
        with ExitStack() as ctx:
            const = ctx.enter_context(tc.tile_pool(name="const", bufs=1))
            gp = ctx.enter_context(tc.tile_pool(name="g", bufs=2))
            h1p = ctx.enter_context(tc.tile_pool(name="h1", bufs=2))
            actp = ctx.enter_context(tc.tile_pool(name="act", bufs=4))
            smp = ctx.enter_context(tc.tile_pool(name="small", bufs=4))
            ps1 = ctx.enter_context(tc.tile_pool(name="ps1", bufs=2, space="PSUM"))
            psT = ctx.enter_context(tc.tile_pool(name="psT", bufs=2, space="PSUM"))
            ps2 = ctx.enter_context(tc.tile_pool(name="ps2", bufs=2, space="PSUM"))
            ps3 = ctx.enter_context(tc.tile_pool(name="ps3", bufs=2, space="PSUM"))

            # ---- resident constants ----
            idx_s = const.tile([P, NG, NWIN, nidx_w // 16], i16)
            nc.sync.dma_start(idx_s[:], idxs.ap())
            # xc split per group so group 0's matmuls don't wait on group 1's bytes
            xc_s = const.tile([P, NG, t_tiles, P], bf16)
            for g in range(NG):
                nc.sync.dma_start(xc_s[:, g], xc.ap()[:, g])
            b1_s = const.tile([P, M_T], f32)
            nc.sync.dma_start(b1_s[:], b1.ap())
            w2t_s = const.tile([P, M_T, H2], f32)
            nc.sync.dma_start(w2t_s[:], w2t.ap())
            b2_s = const.tile([H2, 1], f32)
            nc.sync.dma_start(b2_s[:], b2.ap())
            woutt_s = const.tile([H2 + 1, 1], f32)
            nc.sync.dma_start(woutt_s[:], woutt.ap())
            ident_s = const.tile([P, P], f32)
            nc.sync.dma_start(ident_s[:], ident.ap())

            for g in range(NG):
                # gather the union of active W1T rows for this group, one call
                # per 32768-row window of the table
                gt = gp.tile([P, t_tiles, H1], bf16, name=f"g{g}", tag="g")
                # ucode SWDGE ring holds 1024 descriptors; split each window
                # into sub-calls that fit
                nsub = nidx_w // 2
                sub_slots = nsub // P
                for w in range(NWIN):
                    for s in range(2):
                        s0 = w * slots_w + s * sub_slots
                        nc.gpsimd.dma_gather(
                            gt[:, s0:s0 + sub_slots, :],
                            w1t.ap()[w * WIN:(w + 1) * WIN, :],
                            idx_s[:, g, w, s * (nsub // 16):(s + 1) * (nsub // 16)],
                            nsub,
                            nsub,
                            H1,
                        )

                # fc1: h1[b, d] = sum_t xc[:, t, b].T @ G[:, t, d]
                ps = ps1.tile([P, H1], f32, name=f"ps1_{g}", tag="ps1")
                for t in range(t_tiles):
                    nc.tensor.matmul(
                        ps[:],
                        xc_s[:, g, t, :],
                        gt[:, t, :],
                        start=(t == 0),
                        stop=(t == t_tiles - 1),
                    )
                h1s = h1p.tile([P, H1], f32, name=f"h1s{g}", tag="h1s")
                nc.vector.tensor_copy(h1s[:], ps[:])

                # transpose h1 to [d, b], bias+relu, then fc2/fc3 as usual
                acts = []
                for m in range(M_T):
                    pst = psT.tile([P, P], f32, name=f"psT{g}_{m}", tag="psT")
                    nc.tensor.transpose(
                        pst[:], h1s[:, m * P:(m + 1) * P], ident_s[:]
                    )
                    act = actp.tile([P, P], f32, name=f"act{g}_{m}", tag="act")
                    nc.scalar.activation(
                        act[:], pst[:],
                        mybir.ActivationFunctionType.Relu,
                        bias=b1_s[:, m:m + 1],
                    )
                    acts.append(act)

                p2 = ps2.tile([H2, P], f32, name=f"p2_{g}", tag="p2")
                for m in range(M_T):
                    nc.tensor.matmul(
                        p2[:], w2t_s[:, m, :], acts[m][:],
                        start=(m == 0), stop=(m == M_T - 1),
                    )
                h2t = smp.tile([H2 + 1, P], f32, tag="h2", name=f"h2t{g}")
                nc.scalar.activation(
                    h2t[0:H2, :], p2[:],
                    mybir.ActivationFunctionType.Relu,
                    bias=b2_s[:],
                )
                nc.vector.memset(h2t[H2:H2 + 1, :], 1.0)

                p3 = ps3.tile([1, P], f32, name=f"p3_{g}", tag="p3")
                nc.tensor.matmul(p3[:], woutt_s[:], h2t[:], start=True, stop=True)
                ot = smp.tile([1, P], f32, tag="ot", name=f"ot{g}")
                nc.vector.tensor_copy(ot[:], p3[:])
                nc.sync.dma_start(out.ap()[g, :], ot[:])

    nc.compile()
    return nc


def get_program(nidx_w=NIDX_W):
    key = ("nc", nidx_w)
    if key not in _CACHED:
        _CACHED[key] = _build_program(nidx_w)
    return _CACHED[key]


def _pack_idxs(local, nidx_w):
    """[nidx_w] int16 position-ordered indices -> [P, nidx_w//16] SBUF layout.

    Position i is read from partition i%16, column i//16; the 16-partition
    block is replicated across all 128 partitions.
    """
    arr = local.reshape(nidx_w // 16, 16).T  # [16, cols]
    return np.tile(arr, (8, 1))              # [128, cols]


def _prep_inputs(x, W1, b1, W2, b2, Wout, bout, nidx_w):
    bf = ml_dtypes.bfloat16
    slots_w = nidx_w // P
    t_tiles = NWIN * slots_w

    w1t_h = np.ascontiguousarray(W1.T.astype(bf))                # [98304, 256]
    b1_h = np.ascontiguousarray(b1.reshape(M_T, P).T)            # [P, M_T]
    w2t_h = np.ascontiguousarray(W2.T.reshape(M_T, P, H2).transpose(1, 0, 2))
    b2_h = np.ascontiguousarray(b2.reshape(H2, 1))
    woutt_h = np.concatenate(
        [Wout.T, bout.reshape(1, 1)], axis=0
    ).astype(np.float32)                                         # [H2+1, 1]
    ident_h = np.eye(P, dtype=np.float32)

    rows_all, cols_all = np.nonzero(x != 0.0)

    in_maps = []
    for c in range(N_CORES):
        idx_h = np.zeros((P, NG, NWIN, nidx_w // 16), dtype=np.int16)
        xc_h = np.zeros((NG, t_tiles * P, P), dtype=np.float32)
        for g in range(NG):
            lo = c * RPC + g * P
            sel = (rows_all >= lo) & (rows_all < lo + P)
            bs = (rows_all[sel] - lo).astype(np.int64)
            fs = cols_all[sel].astype(np.int64)
            posmap = np.full(IN_DIM, -1, dtype=np.int64)
            u_all = np.unique(fs)
            for w in range(NWIN):
                uw = u_all[(u_all >= w * WIN) & (u_all < (w + 1) * WIN)]
                n_w = len(uw)
                if n_w > nidx_w:
                    raise OverflowError(n_w)
                local = np.zeros(nidx_w, dtype=np.int16)
                local[:n_w] = (uw - w * WIN).astype(np.int16)
                idx_h[:, g, w, :] = _pack_idxs(local, nidx_w)
                j = np.arange(n_w)
                posmap[uw] = (w * slots_w + j // P) * P + (j % P)
            xc_h[g, posmap[fs], bs] = 1.0
        in_maps.append({
            "w1t": w1t_h,
            "idxs": idx_h,
            "xc": np.ascontiguousarray(
                xc_h.reshape(NG, t_tiles, P, P).transpose(2, 0, 1, 3).astype(bf)
            ),
            "b1": b1_h,
            "w2t": w2t_h,
            "b2": b2_h,
            "woutt": woutt_h,
            "ident": ident_h,
        })
    return in_maps


def kernel(x, W1, b1, W2, b2, Wout, bout, _trace=False, _trace_kwargs=None):
    x = np.asarray(x, dtype=np.float32)
    W1 = np.asarray(W1, dtype=np.float32)
    b1 = np.asarray(b1, dtype=np.float32)
    W2 = np.asarray(W2, dtype=np.float32)
    b2 = np.asarray(b2, dtype=np.float32)
    Wout = np.asarray(Wout, dtype=np.float32)
    bout = np.asarray(bout, dtype=np.float32)

    nidx_w = NIDX_W
    while True:
        try:
            in_maps = _prep_inputs(x, W1, b1, W2, b2, Wout, bout, nidx_w)
            break
        except OverflowError as e:
            # denser input than expected: grow the per-window capacity
            nidx_w = ((int(e.args[0]) + P - 1) // P + 1) * P

    nc = get_program(nidx_w)
    res = run_bass_kernel_spmd(
        nc,
        in_maps,
        core_ids=list(range(N_CORES)),
        trace=_trace,
        **(_trace_kwargs or {}),
    )
    out = np.concatenate(
        [res.results[c]["out"].reshape(RPC) for c in range(N_CORES)]
    ).reshape(B, 1).astype(np.float32)
    if _trace:
        kernel.last_results = res
    return out


if __name__ == "__main__":
    rng = np.random.default_rng(0)
    x = (rng.random((B, IN_DIM)) < 32.0 / IN_DIM).astype(np.float32)
    W1 = rng.standard_normal((H1, IN_DIM), dtype=np.float32) / np.sqrt(IN_DIM)
    b1 = rng.standard_normal(H1, dtype=np.float32) / np.sqrt(IN_DIM)
    W2 = rng.standard_normal((H2, H1), dtype=np.float32) / np.sqrt(H1)
    b2 = rng.standard_normal(H2, dtype=np.float32) / np.sqrt(H1)
    Wout = rng.standard_normal((1, H2), dtype=np.float32) / np.sqrt(H2)
    bout = rng.standard_normal(1, dtype=np.float32) / np.sqrt(H2)
    got = kernel(x, W1, b1, W2, b2, Wout, bout)
    h1 = np.maximum(x @ W1.T + b1, 0)
    h2 = np.maximum(h1 @ W2.T + b2, 0)
    exp = h2 @ Wout.T + bout
    print("rel err:", np.abs(got - exp).max() / np.abs(exp).max())


# revision 7
# speedup vs baseline: 1.8492x; 1.8492x over previous
"""HalfKA NNUE forward pass on 8 Trainium2 NeuronCores — sparse gather version.

Network (fp32 reference):
    h1  = relu(x @ W1.T + b1)     x:[2048, 98304] sparse 0/1 (~32 nnz/row), W1:[256, 98304]
    h2  = relu(h1 @ W2.T + b2)    W2:[32, 256]
    out = h2 @ Wout.T + bout      Wout:[1, 32]  -> [2048, 1]

Strategy: data-parallel over the batch. Each core handles 256 rows as two
groups of 128. Instead of streaming the dense x (100 MB/core), the host
extracts the active-feature indices; the device gathers only the needed W1.T
rows (bf16, ~4 MB/core) with gpsimd.dma_gather and contracts them against a
host-built 0/1 selection matrix xc on the PE:

    h1[b, :] = sum_u xc[u, b] * W1T[U[u], :]      (U = union of the group's
                                                   active features)

dma_gather uses int16 indices (max 32767 < 98304), so each group's union is
split into 3 windows of 32768 rows, one gather call per window with a
base-offset view of the table. Pad slots point at the window's row 0; the
zeros in xc kill their contribution. fc2/fc3 are tiny and computed per group.
No collectives: each core writes its own 256 outputs.
"""

import sys

sys.path.insert(0, "/opt/trn_rl_repo")

from contextlib import ExitStack

import numpy as np
import ml_dtypes

import concourse.bass as bass
import concourse.tile as tile
from concourse import bacc, mybir
from concourse.bass_utils import run_bass_kernel_spmd

f32 = mybir.dt.float32
bf16 = mybir.dt.bfloat16
i16 = mybir.dt.int16

N_CORES = 8
B = 2048
IN_DIM = 98304
H1 = 256
H2 = 32
P = 128

RPC = B // N_CORES               # 256 rows per core
NG = RPC // P                    # 2 groups of 128 rows per core
NWIN = 3                         # int16 index windows over IN_DIM
WIN = 32768
NIDX_W = 1536                    # gathered-index capacity per (group, window)
SLOTS_W = NIDX_W // P            # 12 k-slots per window
T = NWIN * SLOTS_W               # 36 k-tiles per group
M_T = H1 // P                    # 2 h1 partition-tiles

_CACHED = {}


def _build_program(nidx_w=NIDX_W):
    slots_w = nidx_w // P
    t_tiles = NWIN * slots_w

    nc = bacc.Bacc(
        "TRN2",
        target_bir_lowering=False,
        debug=False,
        num_devices=N_CORES,
        num_swdge_queues=4,
    )

    w1t = nc.dram_tensor("w1t", [IN_DIM, H1], bf16, kind="ExternalInput")
    idxs = nc.dram_tensor("idxs", [P, NG, NWIN, nidx_w // 16], i16, kind="ExternalInput")
    xc = nc.dram_tensor("xc", [P, NG, t_tiles, P], bf16, kind="ExternalInput")
    b1 = nc.dram_tensor("b1", [P, M_T], f32, kind="ExternalInput")
    w2t = nc.dram_tensor("w2t", [P, M_T, H2], f32, kind="ExternalInput")
    b2 = nc.dram_tensor("b2", [H2, 1], f32, kind="ExternalInput")
    woutt = nc.dram_tensor("woutt", [H2 + 1, 1], f32, kind="ExternalInput")
    ident = nc.dram_tensor("ident", [P, P], f32, kind="ExternalInput")
    out = nc.dram_tensor("out", [NG, P], f32, kind="ExternalOutput")

    with tile.TileContext(nc) as tc:
        with ExitStack() as ctx:
            const = ctx.enter_context(tc.tile_pool(name="const", bufs=1))
            gp = ctx.enter_context(tc.tile_pool(name="g", bufs=2))
            h1p = ctx.enter_context(tc.tile_pool(name="h1", bufs=2))
            actp = ctx.enter_context(tc.tile_pool(name="act", bufs=4))
            smp = ctx.enter_context(tc.tile_pool(name="small", bufs=4))
            ps1 = ctx.enter_context(tc.tile_pool(name="ps1", bufs=2, space="PSUM"))
            psT = ctx.enter_context(tc.tile_pool(name="psT", bufs=2, space="PSUM"))
            ps2 = ctx.enter_context(tc.tile_pool(name="ps2", bufs=2, space="PSUM"))
            ps3 = ctx.enter_context(tc.tile_pool(name="ps3", bufs=2, space="PSUM"))

            # ---- resident constants ----
            idx_s = const.tile([P, NG, NWIN, nidx_w // 16], i16)
            nc.sync.dma_start(idx_s[:], idxs.ap())
            # xc split per window so early matmuls don't wait on later bytes
            xc_s = const.tile([P, NG, t_tiles, P], bf16)
            for g in range(NG):
                for w in range(NWIN):
                    nc.sync.dma_start(
                        xc_s[:, g, w * slots_w:(w + 1) * slots_w],
                        xc.ap()[:, g, w * slots_w:(w + 1) * slots_w],
                    )
            b1_s = const.tile([P, M_T], f32)
            nc.sync.dma_start(b1_s[:], b1.ap())
            w2t_s = const.tile([P, M_T, H2], f32)
            nc.sync.dma_start(w2t_s[:], w2t.ap())
            b2_s = const.tile([H2, 1], f32)
            nc.sync.dma_start(b2_s[:], b2.ap())
            woutt_s = const.tile([H2 + 1, 1], f32)
            nc.sync.dma_start(woutt_s[:], woutt.ap())
            ident_s = const.tile([P, P], f32)
            nc.sync.dma_start(ident_s[:], ident.ap())

            for g in range(NG):
                # gather the union of active W1T rows for this group, one call
                # per 32768-row window of the table
                gt = gp.tile([P, t_tiles, H1], bf16, name=f"g{g}", tag="g")
                # ucode SWDGE ring holds 1024 descriptors; split each window
                # into sub-calls that fit
                nsub = nidx_w // 2
                sub_slots = nsub // P
                for w in range(NWIN):
                    for s in range(2):
                        s0 = w * slots_w + s * sub_slots
                        qn = (g * NWIN * 2 + w * 2 + s) % 4
                        nc.gpsimd.dma_gather(
                            gt[:, s0:s0 + sub_slots, :],
                            w1t.ap()[w * WIN:(w + 1) * WIN, :],
                            idx_s[:, g, w, s * (nsub // 16):(s + 1) * (nsub // 16)],
                            nsub,
                            nsub,
                            H1,
                            queue_num=qn,
                        )

                # fc1: h1[b, d] = sum_t xc[:, t, b].T @ G[:, t, d]
                ps = ps1.tile([P, H1], f32, name=f"ps1_{g}", tag="ps1")
                for t in range(t_tiles):
                    nc.tensor.matmul(
                        ps[:],
                        xc_s[:, g, t, :],
                        gt[:, t, :],
                        start=(t == 0),
                        stop=(t == t_tiles - 1),
                    )
                h1s = h1p.tile([P, H1], f32, name=f"h1s{g}", tag="h1s")
                nc.vector.tensor_copy(h1s[:], ps[:])

                # transpose h1 to [d, b], bias+relu, then fc2/fc3 as usual
                acts = []
                for m in range(M_T):
                    pst = psT.tile([P, P], f32, name=f"psT{g}_{m}", tag="psT")
                    nc.tensor.transpose(
                        pst[:], h1s[:, m * P:(m + 1) * P], ident_s[:]
                    )
                    act = actp.tile([P, P], f32, name=f"act{g}_{m}", tag="act")
                    nc.scalar.activation(
                        act[:], pst[:],
                        mybir.ActivationFunctionType.Relu,
                        bias=b1_s[:, m:m + 1],
                    )
                    acts.append(act)

                p2 = ps2.tile([H2, P], f32, name=f"p2_{g}", tag="p2")
                for m in range(M_T):
                    nc.tensor.matmul(
                        p2[:], w2t_s[:, m, :], acts[m][:],
                        start=(m == 0), stop=(m == M_T - 1),
                    )
                h2t = smp.tile([H2 + 1, P], f32, tag="h2", name=f"h2t{g}")
                nc.scalar.activation(
                    h2t[0:H2, :], p2[:],
                    mybir.ActivationFunctionType.Relu,
                    bias=b2_s[:],
                )
                nc.vector.memset(h2t[H2:H2 + 1, :], 1.0)

                p3 = ps3.tile([1, P], f32, name=f"p3_{g}", tag="p3")
                nc.tensor.matmul(p3[:], woutt_s[:], h2t[:], start=True, stop=True)
                ot = smp.tile([1, P], f32, tag="ot", name=f"ot{g}")
                nc.vector.tensor_copy(ot[:], p3[:])
                nc.sync.dma_start(out.ap()[g, :], ot[:])

    nc.compile()
    return nc


def get_program(nidx_w=NIDX_W):
    key = ("nc", nidx_w)
    if key not in _CACHED:
        _CACHED[key] = _build_program(nidx_w)
    return _CACHED[key]


def _pack_idxs(local, nidx_w):
    """[nidx_w] int16 position-ordered indices -> [P, nidx_w//16] SBUF layout.

    Position i is read from partition i%16, column i//16; the 16-partition
    block is replicated across all 128 partitions.
    """
    arr = local.reshape(nidx_w // 16, 16).T  # [16, cols]
    return np.tile(arr, (8, 1))              # [128, cols]


def _prep_inputs(x, W1, b1, W2, b2, Wout, bout, nidx_w):
    bf = ml_dtypes.bfloat16
    slots_w = nidx_w // P
    t_tiles = NWIN * slots_w

    w1t_h = np.ascontiguousarray(W1.T.astype(bf))                # [98304, 256]
    b1_h = np.ascontiguousarray(b1.reshape(M_T, P).T)            # [P, M_T]
    w2t_h = np.ascontiguousarray(W2.T.reshape(M_T, P, H2).transpose(1, 0, 2))
    b2_h = np.ascontiguousarray(b2.reshape(H2, 1))
    woutt_h = np.concatenate(
        [Wout.T, bout.reshape(1, 1)], axis=0
    ).astype(np.float32)                                         # [H2+1, 1]
    ident_h = np.eye(P, dtype=np.float32)

    rows_all, cols_all = np.nonzero(x != 0.0)

    in_maps = []
    for c in range(N_CORES):
        idx_h = np.zeros((P, NG, NWIN, nidx_w // 16), dtype=np.int16)
        xc_h = np.zeros((NG, t_tiles * P, P), dtype=np.float32)
        for g in range(NG):
            lo = c * RPC + g * P
            sel = (rows_all >= lo) & (rows_all < lo + P)
            bs = (rows_all[sel] - lo).astype(np.int64)
            fs = cols_all[sel].astype(np.int64)
            posmap = np.full(IN_DIM, -1, dtype=np.int64)
            u_all = np.unique(fs)
            for w in range(NWIN):
                uw = u_all[(u_all >= w * WIN) & (u_all < (w + 1) * WIN)]
                n_w = len(uw)
                if n_w > nidx_w:
                    raise OverflowError(n_w)
                local = np.zeros(nidx_w, dtype=np.int16)
                local[:n_w] = (uw - w * WIN).astype(np.int16)
                idx_h[:, g, w, :] = _pack_idxs(local, nidx_w)
                j = np.arange(n_w)
                posmap[uw] = (w * slots_w + j // P) * P + (j % P)
            xc_h[g, posmap[fs], bs] = 1.0
        in_maps.append({
            "w1t": w1t_h,
            "idxs": idx_h,
            "xc": np.ascontiguousarray(
                xc_h.reshape(NG, t_tiles, P, P).transpose(2, 0, 1, 3).astype(bf)
            ),
            "b1": b1_h,
            "w2t": w2t_h,
            "b2": b2_h,
            "woutt": woutt_h,
            "ident": ident_h,
        })
    return in_maps


def kernel(x, W1, b1, W2, b2, Wout, bout, _trace=False, _trace_kwargs=None):
    x = np.asarray(x, dtype=np.float32)
    W1 = np.asarray(W1, dtype=np.float32)
    b1 = np.asarray(b1, dtype=np.float32)
    W2 = np.asarray(W2, dtype=np.float32)
    b2 = np.asarray(b2, dtype=np.float32)
    Wout = np.asarray(Wout, dtype=np.float32)
    bout = np.asarray(bout, dtype=np.float32)

    nidx_w = NIDX_W
    while True:
        try:
            in_maps = _prep_inputs(x, W1, b1, W2, b2, Wout, bout, nidx_w)
            break
        except OverflowError as e:
            # denser input than expected: grow the per-window capacity
            nidx_w = ((int(e.args[0]) + P - 1) // P + 1) * P

    nc = get_program(nidx_w)
    res = run_bass_kernel_spmd(
        nc,
        in_maps,
        core_ids=list(range(N_CORES)),
        trace=_trace,
        **(_trace_kwargs or {}),
    )
    out = np.concatenate(
        [res.results[c]["out"].reshape(RPC) for c in range(N_CORES)]
    ).reshape(B, 1).astype(np.float32)
    if _trace:
        kernel.last_results = res
    return out


if __name__ == "__main__":
    rng = np.random.default_rng(0)
    x = (rng.random((B, IN_DIM)) < 32.0 / IN_DIM).astype(np.float32)
    W1 = rng.standard_normal((H1, IN_DIM), dtype=np.float32) / np.sqrt(IN_DIM)
    b1 = rng.standard_normal(H1, dtype=np.float32) / np.sqrt(IN_DIM)
    W2 = rng.standard_normal((H2, H1), dtype=np.float32) / np.sqrt(H1)
    b2 = rng.standard_normal(H2, dtype=np.float32) / np.sqrt(H1)
    Wout = rng.standard_normal((1, H2), dtype=np.float32) / np.sqrt(H2)
    bout = rng.standard_normal(1, dtype=np.float32) / np.sqrt(H2)
    got = kernel(x, W1, b1, W2, b2, Wout, bout)
    h1 = np.maximum(x @ W1.T + b1, 0)
    h2 = np.maximum(h1 @ W2.T + b2, 0)
    exp = h2 @ Wout.T + bout
    print("rel err:", np.abs(got - exp).max() / np.abs(exp).max())


# revision 9
# speedup vs baseline: 1.8612x; 1.0065x over previous
"""HalfKA NNUE forward pass on 8 Trainium2 NeuronCores — sparse gather version.

Network (fp32 reference):
    h1  = relu(x @ W1.T + b1)     x:[2048, 98304] sparse 0/1 (~32 nnz/row), W1:[256, 98304]
    h2  = relu(h1 @ W2.T + b2)    W2:[32, 256]
    out = h2 @ Wout.T + bout      Wout:[1, 32]  -> [2048, 1]

Strategy: data-parallel over the batch; each core handles 256 rows. Instead
of streaming the dense x (100 MB/core), the host extracts the active-feature
indices; the device gathers only the needed W1.T rows (bf16, ~4 MB/core) with
gpsimd.dma_gather and contracts them against a host-built 0/1 selection
matrix xc on the PE:

    h1.T[d, b] = sum_u W1T[U[u], d] * xc[u, b]    (U = union of the core's
                                                   active features)

dma_gather uses int16 indices (max 32767 < 98304), so the union is split into
3 windows of 32768 rows with a base-offset view of the table per window. The
SWDGE ring holds 1024 descriptors, so each window is gathered in sub-calls
(<=768 idx) spread over 4 SWDGE queues whose descriptor generation runs
concurrently. Pad slots point at the window's row 0; zeros in xc kill their
contribution. fc2/fc3 are tiny. No collectives: each core writes its own 256
outputs.
"""

import sys

sys.path.insert(0, "/opt/trn_rl_repo")

from contextlib import ExitStack

import numpy as np
import ml_dtypes

import concourse.bass as bass
import concourse.tile as tile
from concourse import bacc, mybir
from concourse.bass_utils import run_bass_kernel_spmd

f32 = mybir.dt.float32
bf16 = mybir.dt.bfloat16
i16 = mybir.dt.int16

N_CORES = 8
B = 2048
IN_DIM = 98304
H1 = 256
H2 = 32
P = 128

RPC = B // N_CORES               # 256 rows per core
NWIN = 3                         # int16 index windows over IN_DIM
WIN = 32768
CAP_W = 2816                     # gathered-index capacity per window (22 slots)
# sub-call sizes per window (must each be a multiple of 128 and sum to CAP_W;
# first one small so the PE can start early)
SUBS = [256, 768, 768, 768, 256]
SLOTS_W = CAP_W // P             # 22
T = NWIN * SLOTS_W               # 66 k-tiles
M_T = H1 // P                    # 2 h1 partition-tiles

_CACHED = {}


def _build_program(cap_w=CAP_W, subs=tuple(SUBS)):
    slots_w = cap_w // P
    t_tiles = NWIN * slots_w
    assert sum(subs) == cap_w and all(s % P == 0 for s in subs)

    nc = bacc.Bacc(
        "TRN2",
        target_bir_lowering=False,
        debug=False,
        num_devices=N_CORES,
        num_swdge_queues=4,
    )

    w1t = nc.dram_tensor("w1t", [IN_DIM, H1], bf16, kind="ExternalInput")
    idxs = nc.dram_tensor("idxs", [P, NWIN, cap_w // 16], i16, kind="ExternalInput")
    xc = nc.dram_tensor("xc", [P, t_tiles, RPC], bf16, kind="ExternalInput")
    b1 = nc.dram_tensor("b1", [P, M_T], f32, kind="ExternalInput")
    w2t = nc.dram_tensor("w2t", [P, M_T, H2], f32, kind="ExternalInput")
    b2 = nc.dram_tensor("b2", [H2, 1], f32, kind="ExternalInput")
    woutt = nc.dram_tensor("woutt", [H2 + 1, 1], f32, kind="ExternalInput")
    out = nc.dram_tensor("out", [1, RPC], f32, kind="ExternalOutput")

    with tile.TileContext(nc) as tc:
        with ExitStack() as ctx:
            const = ctx.enter_context(tc.tile_pool(name="const", bufs=1))
            gp = ctx.enter_context(tc.tile_pool(name="g", bufs=1))
            actp = ctx.enter_context(tc.tile_pool(name="act", bufs=2))
            smp = ctx.enter_context(tc.tile_pool(name="small", bufs=4))
            ps1 = ctx.enter_context(tc.tile_pool(name="ps1", bufs=2, space="PSUM"))
            ps2 = ctx.enter_context(tc.tile_pool(name="ps2", bufs=2, space="PSUM"))
            ps3 = ctx.enter_context(tc.tile_pool(name="ps3", bufs=2, space="PSUM"))

            # idx load goes first and alone on the sync DMA queue: the first
            # gather depends only on it
            idx_s = const.tile([P, NWIN, cap_w // 16], i16)
            nc.sync.dma_start(idx_s[:], idxs.ap())

            # gathers: one sub-call chain per window, rotating SWDGE queues
            gt = gp.tile([P, t_tiles, H1], bf16, name="g", tag="g")
            qn = 0
            for w in range(NWIN):
                pos = 0
                for s in subs:
                    s0 = w * slots_w + pos // P
                    nc.gpsimd.dma_gather(
                        gt[:, s0:s0 + s // P, :],
                        w1t.ap()[w * WIN:(w + 1) * WIN, :],
                        idx_s[:, w, pos // 16:(pos + s) // 16],
                        s,
                        s,
                        H1,
                        queue_num=qn % 4,
                    )
                    qn += 1
                    pos += s

            # xc + small constants on the scalar/vector DMA queues so they
            # don't delay the idx load the gathers wait on
            xc_s = const.tile([P, t_tiles, RPC], bf16)
            for w in range(NWIN):
                sl = slice(w * slots_w, (w + 1) * slots_w)
                nc.scalar.dma_start(xc_s[:, sl], xc.ap()[:, sl])
            b1_s = const.tile([P, M_T], f32)
            nc.scalar.dma_start(b1_s[:], b1.ap())
            w2t_s = const.tile([P, M_T, H2], f32)
            nc.scalar.dma_start(w2t_s[:], w2t.ap())
            b2_s = const.tile([H2, 1], f32)
            nc.scalar.dma_start(b2_s[:], b2.ap())
            woutt_s = const.tile([H2 + 1, 1], f32)
            nc.scalar.dma_start(woutt_s[:], woutt.ap())

            # fc1: h1T[m][d, b] = sum_t G[:, t, m-slice].T @ xc[:, t, :]
            psum_m = [
                ps1.tile([P, RPC], f32, tag=f"ps1_{m}", name=f"ps1m{m}")
                for m in range(M_T)
            ]
            for t in range(t_tiles):
                for m in range(M_T):
                    nc.tensor.matmul(
                        psum_m[m][:],
                        gt[:, t, m * P:(m + 1) * P],
                        xc_s[:, t, :],
                        start=(t == 0),
                        stop=(t == t_tiles - 1),
                    )

            # bias+relu straight out of PSUM, then fc2/fc3
            acts = []
            for m in range(M_T):
                act = actp.tile([P, RPC], f32, name=f"act{m}", tag="act")
                nc.scalar.activation(
                    act[:], psum_m[m][:],
                    mybir.ActivationFunctionType.Relu,
                    bias=b1_s[:, m:m + 1],
                )
                acts.append(act)

            p2 = ps2.tile([H2, RPC], f32, name="p2", tag="p2")
            for m in range(M_T):
                nc.tensor.matmul(
                    p2[:], w2t_s[:, m, :], acts[m][:],
                    start=(m == 0), stop=(m == M_T - 1),
                )
            h2t = smp.tile([H2 + 1, RPC], f32, tag="h2", name="h2t")
            nc.scalar.activation(
                h2t[0:H2, :], p2[:],
                mybir.ActivationFunctionType.Relu,
                bias=b2_s[:],
            )
            nc.vector.memset(h2t[H2:H2 + 1, :], 1.0)

            p3 = ps3.tile([1, RPC], f32, name="p3", tag="p3")
            nc.tensor.matmul(p3[:], woutt_s[:], h2t[:], start=True, stop=True)
            ot = smp.tile([1, RPC], f32, tag="ot", name="ot")
            nc.vector.tensor_copy(ot[:], p3[:])
            nc.sync.dma_start(out.ap()[0, :], ot[:])

    nc.compile()
    return nc


def get_program(cap_w=CAP_W):
    key = ("nc", cap_w)
    if key not in _CACHED:
        subs = SUBS if cap_w == CAP_W else [768] * (cap_w // 768) + (
            [cap_w % 768] if cap_w % 768 else []
        )
        _CACHED[key] = _build_program(cap_w, tuple(subs))
    return _CACHED[key]


def _pack_idxs(local, cap_w):
    """[cap_w] int16 position-ordered indices -> [P, cap_w//16] SBUF layout.

    Position i is read from partition i%16, column i//16; the 16-partition
    block is replicated across all 128 partitions.
    """
    arr = local.reshape(cap_w // 16, 16).T  # [16, cols]
    return np.tile(arr, (8, 1))             # [128, cols]


def _prep_inputs(x, W1, b1, W2, b2, Wout, bout, cap_w):
    bf = ml_dtypes.bfloat16
    slots_w = cap_w // P
    t_tiles = NWIN * slots_w

    w1t_h = np.ascontiguousarray(W1.T.astype(bf))                # [98304, 256]
    b1_h = np.ascontiguousarray(b1.reshape(M_T, P).T)            # [P, M_T]
    w2t_h = np.ascontiguousarray(W2.T.reshape(M_T, P, H2).transpose(1, 0, 2))
    b2_h = np.ascontiguousarray(b2.reshape(H2, 1))
    woutt_h = np.concatenate(
        [Wout.T, bout.reshape(1, 1)], axis=0
    ).astype(np.float32)                                         # [H2+1, 1]

    rows_all, cols_all = np.nonzero(x != 0.0)

    in_maps = []
    for c in range(N_CORES):
        lo = c * RPC
        sel = (rows_all >= lo) & (rows_all < lo + RPC)
        bs = (rows_all[sel] - lo).astype(np.int64)
        fs = cols_all[sel].astype(np.int64)
        posmap = np.full(IN_DIM, -1, dtype=np.int64)
        u_all = np.unique(fs)
        idx_h = np.zeros((P, NWIN, cap_w // 16), dtype=np.int16)
        for w in range(NWIN):
            uw = u_all[(u_all >= w * WIN) & (u_all < (w + 1) * WIN)]
            n_w = len(uw)
            if n_w > cap_w:
                raise OverflowError(n_w)
            local = np.zeros(cap_w, dtype=np.int16)
            local[:n_w] = (uw - w * WIN).astype(np.int16)
            idx_h[:, w, :] = _pack_idxs(local, cap_w)
            j = np.arange(n_w)
            posmap[uw] = (w * slots_w + j // P) * P + (j % P)
        xc_h = np.zeros((t_tiles * P, RPC), dtype=np.float32)
        xc_h[posmap[fs], bs] = 1.0
        in_maps.append({
            "w1t": w1t_h,
            "idxs": idx_h,
            "xc": np.ascontiguousarray(
                xc_h.reshape(t_tiles, P, RPC).transpose(1, 0, 2).astype(bf)
            ),
            "b1": b1_h,
            "w2t": w2t_h,
            "b2": b2_h,
            "woutt": woutt_h,
        })
    return in_maps


def kernel(x, W1, b1, W2, b2, Wout, bout, _trace=False, _trace_kwargs=None):
    x = np.asarray(x, dtype=np.float32)
    W1 = np.asarray(W1, dtype=np.float32)
    b1 = np.asarray(b1, dtype=np.float32)
    W2 = np.asarray(W2, dtype=np.float32)
    b2 = np.asarray(b2, dtype=np.float32)
    Wout = np.asarray(Wout, dtype=np.float32)
    bout = np.asarray(bout, dtype=np.float32)

    cap_w = CAP_W
    while True:
        try:
            in_maps = _prep_inputs(x, W1, b1, W2, b2, Wout, bout, cap_w)
            break
        except OverflowError as e:
            # denser input than expected: grow the per-window capacity
            cap_w = ((int(e.args[0]) + P - 1) // P + 1) * P

    nc = get_program(cap_w)
    res = run_bass_kernel_spmd(
        nc,
        in_maps,
        core_ids=list(range(N_CORES)),
        trace=_trace,
        **(_trace_kwargs or {}),
    )
    out = np.concatenate(
        [res.results[c]["out"].reshape(RPC) for c in range(N_CORES)]
    ).reshape(B, 1).astype(np.float32)
    if _trace:
        kernel.last_results = res
    return out


if __name__ == "__main__":
    rng = np.random.default_rng(0)
    x = (rng.random((B, IN_DIM)) < 32.0 / IN_DIM).astype(np.float32)
    W1 = rng.standard_normal((H1, IN_DIM), dtype=np.float32) / np.sqrt(IN_DIM)
    b1 = rng.standard_normal(H1, dtype=np.float32) / np.sqrt(IN_DIM)
    W2 = rng.standard_normal((H2, H1), dtype=np.float32) / np.sqrt(H1)
    b2 = rng.standard_normal(H2, dtype=np.float32) / np.sqrt(H1)
    Wout = rng.standard_normal((1, H2), dtype=np.float32) / np.sqrt(H2)
    bout = rng.standard_normal(1, dtype=np.float32) / np.sqrt(H2)
    got = kernel(x, W1, b1, W2, b2, Wout, bout)
    h1 = np.maximum(x @ W1.T + b1, 0)
    h2 = np.maximum(h1 @ W2.T + b2, 0)
    exp = h2 @ Wout.T + bout
    print("rel err:", np.abs(got - exp).max() / np.abs(exp).max())
